# revision 25
# baseline (speedup 1.0000x reference)
"""ArcMarginLoss distributed Trainium2 kernel (8 NeuronCores, class-sharded).

Math (equivalent to the reference, no arccos needed):
  x_hat = x / max(||x||, eps);  w_hat = w / max(||w||, eps)
  cos[i,c] = x_hat[i] . w_hat[c]
  For the label class only: m_i = cos(arccos(clip(c_i)) + M)
                                = clip(c_i)*cos(M) - sin(M)*sqrt(1-clip(c_i)^2)
  logits = S*cos except S*m_i at the label
  nll_i = logsumexp_c(logits[i]) - S*m_i
        = ln( sum_c exp(S*cos[i,c]) - exp(S*c_i) + exp(S*m_i) ) - S*m_i
  out = mean_i nll_i
S*cos is in [-16, 16] so no max-subtraction is needed for a stable sum-exp.

Distribution: classes padded 32000 -> 32768, 4096 per core.  Each core
computes its local sum-exp plus its owned rows' correction terms; two
[128,64] f32 AllReduces combine A[i] (sum-exp) and B[i] = S*m_i, then every
core computes mean(ln(A - 768) - B).

Implementation notes:
- matmul runs in fp8e4 with perf_mode=DoubleRow (2 fp8 weights per PE cell,
  ~2x bf16 throughput at free-dim 512): x is cast raw f32->fp8 (values
  ~N(0,1)); w is normalized and scaled by 32 before the fp8 cast so the
  values sit in fp8's normal range.  The psum accumulates fp32 and exp
  folds S/(32*||x||) in as a per-row activation scale, so overall rel-err
  stays ~2e-5.
- both x and w arrive ALSO transposed from the host ([D, N] / [D, CS]
  layout views) so the kernel needs no on-chip transposes at all.  The
  per-class norm scale is computed from the row-major w, bounced through
  DRAM, and broadcast-read back to all 128 partitions ([P, CS] tile), so
  the scale-and-cast of transposed w is a plain DVE multiply.
- the label-cosine path gathers raw w rows (indirect DMA) + the per-class
  scale from the DRAM bounce buffer, and dots against the row-major f32 x
  tiles on the DVE.
- sum-exp per 128-row tile: two [128, 2048] psum slots, ACT exp with
  accum_out; on every 4th row-tile the first slot is instead computed on
  the DVE with a Schraudolph fast-exp (scale+bias -> int32 -> bitcast f32
  -> accum), balancing the ACT and DVE engine loads.
- the activation-table cache is pre-seeded so Ln and Exp both resolve to
  the combined natural_log_exp set (one table load instead of ~28).
"""

import math
import sys

sys.path.insert(0, "/opt/trn_rl_repo")

import numpy as np

from concourse import bacc, bass, mybir, tile
from concourse.bass_utils import run_bass_kernel_spmd

f32 = mybir.dt.float32
bf16 = mybir.dt.bfloat16
fp8 = mybir.dt.float8e4
i32 = mybir.dt.int32

N, D, C = 8192, 512, 32000
NCORES = 8
CPAD = 32768            # padded class count (8 * 4096)
CS = CPAD // NCORES     # classes per core
P = 128                 # partitions
RT = N // P             # row tiles (64)
WGS = 8                 # w sumsq groups (512 classes each)
HB = RT // 2            # half of the row-tile columns
NPAD = float(CPAD - C)  # zero-pad classes, each contributes exp(0)=1

S_SCALE = 16.0
M_MARGIN = 0.2
EPS = 1e-7
COS_M = math.cos(M_MARGIN)
LN_SIN_M = math.log(math.sin(M_MARGIN))
SS_FLOOR = 1e-24        # max(ss, floor) emulates torch F.normalize eps=1e-12
FP8S = 32.0             # scale factor on normalized w before the fp8 cast
LN_S_FP8S = math.log(S_SCALE / FP8S)
LN_FP8S = math.log(FP8S)
FEXP_A = 2.0 ** 23 / math.log(2.0)       # fast-exp multiplier
FEXP_B = (127.0 - 0.0430) * 2.0 ** 23    # fast-exp bias (Schraudolph)
FE_EVERY = 4            # every FE_EVERY-th row-tile's first slot on the DVE

_CACHE = {}


def _patch_act_tables():
    """Make Ln and Exp resolve to the combined natural_log_exp set so the
    table-load pass emits one load instead of thrashing between sets."""
    from concourse.hw_specs import get_activation_tables

    Exp = mybir.ActivationFunctionType.Exp
    Ln = mybir.ActivationFunctionType.Ln
    tabs = get_activation_tables("gen3")   # cached dict, mutate in place
    combined = [n for n, fns in tabs.items() if Exp in fns and Ln in fns]
    if not combined:
        return
    keep = combined[0]
    for name, fns in tabs.items():
        if name != keep:
            fns.discard(Exp)
            fns.discard(Ln)


def _build(ncores=NCORES):
    _patch_act_tables()
    nc = bacc.Bacc("TRN2", target_bir_lowering=False, debug=False,
                   num_devices=ncores)
    x_d = nc.dram_tensor("x", [N, D], f32, kind="ExternalInput")
    xT_d = nc.dram_tensor("xT", [D, N], f32, kind="ExternalInput")
    w_d = nc.dram_tensor("w", [CS, D], f32, kind="ExternalInput")
    wT_d = nc.dram_tensor("wT", [D, CS], f32, kind="ExternalInput")
    lab_d = nc.dram_tensor("lab", [P, RT], i32, kind="ExternalInput")
    msk_d = nc.dram_tensor("msk", [P, RT], f32, kind="ExternalInput")
    out_d = nc.dram_tensor("out", [1, 1], f32, kind="ExternalOutput")

    mult = mybir.AluOpType.mult
    add = mybir.AluOpType.add
    sub = mybir.AluOpType.subtract
    amax = mybir.AluOpType.max
    amin = mybir.AluOpType.min
    Exp = mybir.ActivationFunctionType.Exp
    Ln = mybir.ActivationFunctionType.Ln
    DR = mybir.MatmulPerfMode.DoubleRow

    with tile.TileContext(nc) as tc:
        with tc.tile_pool(name="persist", bufs=1) as persist, \
             tc.tile_pool(name="dram", bufs=1, space="DRAM") as dram, \
             tc.tile_pool(name="xts", bufs=4) as xts, \
             tc.tile_pool(name="wts", bufs=4) as wts, \
             tc.tile_pool(name="xrow", bufs=4) as xrow, \
             tc.tile_pool(name="xbig", bufs=2) as xbig, \
             tc.tile_pool(name="wsum", bufs=3) as wsum, \
             tc.tile_pool(name="ej", bufs=2) as ejp, \
             tc.tile_pool(name="junk", bufs=2) as junkp, \
             tc.tile_pool(name="small", bufs=4) as small, \
             tc.tile_pool(name="pmm", bufs=2, space="PSUM") as pmm:

            # --- persistent tiles ---
            def T(shape, name, dtype=f32):
                return persist.tile(shape, dtype, name=name)

            ones = T([P, 1], "ones")
            nc.vector.memset(ones[:], 1.0)
            ln_fp8s_c = T([P, 1], "ln_fp8s_c")
            nc.vector.memset(ln_fp8s_c[:], LN_FP8S)
            ln_s_fp8s_c = T([P, 1], "ln_s_fp8s_c")
            nc.vector.memset(ln_s_fp8s_c[:], LN_S_FP8S)
            ln_sin_m_c = T([P, 1], "ln_sin_m_c")
            nc.vector.memset(ln_sin_m_c[:], LN_SIN_M)

            labs = T([P, RT], "labs", dtype=i32)
            nc.gpsimd.dma_start(out=labs[:], in_=lab_d[:, :])
            msks = T([P, RT], "msks")
            nc.gpsimd.dma_start(out=msks[:], in_=msk_d[:, :])

            xT8 = T([P, 4, N], "xT8", dtype=fp8)       # 32KB/part
            whT8 = T([P, 4, CS], "whT8", dtype=fp8)    # 16KB/part
            wscb = T([P, CS], "wscb")                  # 16KB/part

            ssx = T([P, RT], "ssx")
            sxe = T([P, RT], "sxe")      # S / (32*||x_r||), exp scale
            sxf = T([P, RT], "sxf")      # sxe * 2^23/ln2, fast-exp scale
            ssw = T([P, WGS * 4], "ssw")
            wsc = T([P, WGS * 4], "wsc")
            dotg = T([P, RT], "dotg")
            wsls = T([P, RT], "wsls")    # gathered per-label class scale
            accAB = T([P, 2 * RT], "accAB")
            sumexp = T([P, RT], "sumexp")
            cdot = T([P, RT], "cdot")
            ctl = T([P, RT], "ctl")
            marg = T([P, RT], "marg")
            aloc = T([P, RT], "aloc")
            bloc = T([P, RT], "bloc")

            wsc_dram = dram.tile([CS, 1], f32)
            QB = RT // 4
            ar_ins = [dram.tile([P, 2 * QB], f32, name=f"ar_in{q}")
                      for q in range(4)]
            ar_outs = [dram.tile([P, 2 * QB], f32, name=f"ar_out{q}")
                       for q in range(4)]

            # ---- x transposed pieces: [D, N] f32 -> fp8, no transpose ----
            def emit_xT_piece(dc, cc):
                xtp = xts.tile([P, 2048], f32, name="xtp")
                nc.gpsimd.dma_start(
                    out=xtp[:],
                    in_=xT_d[dc * P:(dc + 1) * P, cc * 2048:(cc + 1) * 2048])
                with tc.high_priority():
                    nc.vector.tensor_copy(
                        out=xT8[:, dc, cc * 2048:(cc + 1) * 2048],
                        in_=xtp[:])

            # ---- w norm scales: sumsq from row-major w, bounce through
            #      DRAM, broadcast back to all partitions ----
            def emit_wsum(g):
                wt = wsum.tile([P, 4, D], f32, name="wt")
                # class c = g*512 + p*4 + a  -> 8KB contiguous/partition
                nc.sync.dma_start(
                    out=wt[:],
                    in_=w_d[g * 4 * P:(g + 1) * 4 * P, :].rearrange(
                        "(p a) d -> p a d", a=4))
                with tc.high_priority():
                    for a in range(4):
                        j = g * 4 + a
                        wjk = junkp.tile([P, D], f32, name="junk")
                        nc.vector.scalar_tensor_tensor(
                            out=wjk[:], in0=wt[:, a], scalar=1.0,
                            in1=wt[:, a],
                            op0=mult, op1=mult, accum_out=ssw[:, j:j + 1])
                    gsl = slice(g * 4, (g + 1) * 4)
                    wssc = small.tile([P, 4], f32, name="wssc")
                    nc.vector.tensor_scalar_max(out=wssc[:], in0=ssw[:, gsl],
                                                scalar1=SS_FLOOR)
                    wlns = small.tile([P, 4], f32, name="wlns")
                    nc.scalar.activation(out=wlns[:], in_=wssc[:], func=Ln)
                    # wsc = 32 * ss^-0.5
                    nc.scalar.activation(out=wsc[:, gsl], in_=wlns[:],
                                         func=Exp, scale=-0.5,
                                         bias=ln_fp8s_c[:, :1])
                nc.gpsimd.dma_start(
                    out=wsc_dram[g * 4 * P:(g + 1) * 4 * P, 0].rearrange(
                        "(p a) -> p a", a=4),
                    in_=wsc[:, gsl])
                nc.gpsimd.dma_start(
                    out=wscb[:, g * 512:(g + 1) * 512],
                    in_=wsc_dram[g * 512:(g + 1) * 512, 0].unsqueeze(
                        0).to_broadcast((P, 512)))

            # ---- transposed w: scale (free-dim bcast tile) + fp8 cast ----
            def emit_wT_piece(dc, h):
                wtp = wts.tile([P, 1024], f32, name="wtp")
                nc.gpsimd.dma_start(
                    out=wtp[:],
                    in_=wT_d[dc * P:(dc + 1) * P, h * 1024:(h + 1) * 1024])
                with tc.high_priority():
                    nc.vector.tensor_tensor(
                        out=whT8[:, dc, h * 1024:(h + 1) * 1024],
                        in0=wtp[:], in1=wscb[:, h * 1024:(h + 1) * 1024],
                        op=mult)

            # ---- x row sumsq for ||x||, 4 row-tiles per DMA ----
            def emit_xbig(i):
                xb = xbig.tile([P, 4, D], f32, name="xb")
                # row r = i*512 + a*128 + p: sub-tile a = row-tile 4i+a
                nc.sync.dma_start(
                    out=xb[:],
                    in_=x_d[i * 4 * P:(i + 1) * 4 * P, :].rearrange(
                        "(a p) d -> p a d", a=4))
                with tc.high_priority():
                    for a in range(4):
                        xjk = junkp.tile([P, D], f32, name="junk")
                        nc.vector.scalar_tensor_tensor(
                            out=xjk[:], in0=xb[:, a], scalar=1.0,
                            in1=xb[:, a],
                            op0=mult, op1=mult,
                            accum_out=ssx[:, 4 * i + a:4 * i + a + 1])

            def emit_sxe_batch(b):
                with tc.high_priority():
                    bs = slice(b * 8, (b + 1) * 8)
                    xln = small.tile([P, 8], f32, name="xln")
                    nc.scalar.activation(out=xln[:], in_=ssx[:, bs], func=Ln)
                    nc.scalar.activation(out=sxe[:, bs], in_=xln[:],
                                         func=Exp, scale=-0.5,
                                         bias=ln_s_fp8s_c[:, :1])
                    nc.vector.tensor_scalar_mul(out=sxf[:, bs],
                                                in0=sxe[:, bs],
                                                scalar1=FEXP_A)

            # ---- label gather (raw rows + class scale) + dot ----
            def emit_gather(t):
                wg_t = small.tile([P, D], f32, name="wg")
                nc.gpsimd.indirect_dma_start(
                    out=wg_t[:], out_offset=None, in_=w_d[:, :],
                    in_offset=bass.IndirectOffsetOnAxis(
                        ap=labs[:, t:t + 1], axis=0))
                nc.gpsimd.indirect_dma_start(
                    out=wsls[:, t:t + 1], out_offset=None, in_=wsc_dram[:, :],
                    in_offset=bass.IndirectOffsetOnAxis(
                        ap=labs[:, t:t + 1], axis=0))
                xt2 = xrow.tile([P, D], f32, name="xt2")
                nc.sync.dma_start(out=xt2[:], in_=x_d[t * P:(t + 1) * P, :])
                gjk = junkp.tile([P, D], f32, name="gjunk")
                nc.vector.scalar_tensor_tensor(
                    out=gjk[:], in0=wg_t[:], scalar=1.0, in1=xt2[:],
                    op0=mult, op1=mult, accum_out=dotg[:, t:t + 1])

            # ---- per-row-tile slots: 8 DR matmuls + exp-accumulate ----
            def emit_slot(t, half):
                ps = pmm.tile([P, 2048], f32, name="ps")
                rs = slice(t * P, (t + 1) * P)
                for c in range(4):
                    cg = half * 2048 + c * 512
                    for k in range(2):
                        nc.tensor.matmul(
                            out=ps[:, c * 512:(c + 1) * 512],
                            lhsT=xT8[:, 2 * k:2 * k + 2, rs],
                            rhs=whT8[:, 2 * k:2 * k + 2, cg:cg + 512],
                            start=(k == 0), stop=(k == 1),
                            perf_mode=DR)
                acol = accAB[:, 2 * t + half:2 * t + half + 1]
                if False and half == 0 and t % FE_EVERY == 2:
                    # DVE fast-exp for this slot (engine balancing); high
                    # priority so it never queues behind the dot stream
                    with tc.high_priority(offset=400):
                        bt = bitsp.tile([P, 2048], i32, name="bt")
                        nc.vector.tensor_scalar(
                            out=bt[:], in0=ps[:],
                            scalar1=sxf[:, t:t + 1], scalar2=FEXP_B,
                            op0=mult, op1=add)
                        jb = bitsp.tile([P, 2048], bf16, name="jb")
                        nc.vector.tensor_scalar(
                            out=jb[:], in0=bt[:].bitcast(f32),
                            scalar1=1.0, scalar2=0.0, op0=mult, op1=add,
                            accum_out=acol)
                else:
                    ej = ejp.tile([P, 2048], bf16, name="ej")
                    nc.scalar.activation(
                        out=ej[:], in_=ps[:], func=Exp,
                        scale=sxe[:, t:t + 1], accum_out=acol)

            # ---- corrections + allreduce for one quarter of the rows ----
            def emit_quarter(q):
                cl = slice(q * QB, (q + 1) * QB)
                a2 = accAB[:, 2 * q * QB:2 * (q + 1) * QB].rearrange(
                    "p (t two) -> p two t", two=2)
                nc.vector.tensor_tensor(out=sumexp[:, cl], in0=a2[:, 0],
                                        in1=a2[:, 1], op=add)
                # cos_i = dotg * wsl * sxe / S
                cd = cdot[:, cl]
                nc.vector.tensor_tensor(out=cd, in0=dotg[:, cl],
                                        in1=wsls[:, cl], op=mult)
                nc.vector.scalar_tensor_tensor(
                    out=cd, in0=cd, scalar=1.0 / S_SCALE,
                    in1=sxe[:, cl], op0=mult, op1=mult)
                nc.vector.tensor_scalar(out=ctl[:, cl], in0=cd,
                                        scalar1=(-1.0 + EPS),
                                        scalar2=(1.0 - EPS),
                                        op0=amax, op1=amin)
                negc2 = small.tile([P, QB], f32, name="negc2")
                nc.vector.scalar_tensor_tensor(out=negc2[:], in0=ctl[:, cl],
                                               scalar=-1.0, in1=ctl[:, cl],
                                               op0=mult, op1=mult)
                uu = small.tile([P, QB], f32, name="uu")
                nc.vector.tensor_scalar_add(out=uu[:], in0=negc2[:],
                                            scalar1=1.0)
                lnu = small.tile([P, QB], f32, name="lnu")
                nc.scalar.activation(out=lnu[:], in_=uu[:], func=Ln)
                sinsq = small.tile([P, QB], f32, name="sinsq")
                nc.scalar.activation(out=sinsq[:], in_=lnu[:], func=Exp,
                                     scale=0.5, bias=ln_sin_m_c[:, :1])
                nc.vector.scalar_tensor_tensor(out=marg[:, cl],
                                               in0=ctl[:, cl],
                                               scalar=COS_M, in1=sinsq[:],
                                               op0=mult, op1=sub)
                e1 = small.tile([P, QB], f32, name="e1")
                nc.scalar.activation(out=e1[:], in_=marg[:, cl], func=Exp,
                                     scale=S_SCALE)
                e2 = small.tile([P, QB], f32, name="e2")
                nc.scalar.activation(out=e2[:], in_=ctl[:, cl], func=Exp,
                                     scale=S_SCALE)
                d12 = small.tile([P, QB], f32, name="d12")
                nc.vector.scalar_tensor_tensor(out=d12[:], in0=e1[:],
                                               scalar=1.0, in1=e2[:],
                                               op0=mult, op1=sub)
                corr = small.tile([P, QB], f32, name="corr")
                nc.vector.tensor_tensor(out=corr[:], in0=d12[:],
                                        in1=msks[:, cl], op=mult)
                nc.vector.tensor_tensor(out=aloc[:, cl], in0=sumexp[:, cl],
                                        in1=corr[:], op=add)
                nc.vector.scalar_tensor_tensor(out=bloc[:, cl],
                                               in0=marg[:, cl],
                                               scalar=S_SCALE,
                                               in1=msks[:, cl],
                                               op0=mult, op1=mult)
                ar_i, ar_o = ar_ins[q], ar_outs[q]
                nc.gpsimd.dma_start(out=ar_i[:, 0:QB], in_=aloc[:, cl])
                nc.gpsimd.dma_start(out=ar_i[:, QB:2 * QB], in_=bloc[:, cl])
                nc.gpsimd.collective_compute(
                    "AllReduce", add,
                    replica_groups=[list(range(ncores))],
                    ins=[ar_i[:].opt()], outs=[ar_o[:].opt()])

            # ================= emission schedule =================
            # DVE order tracks the critical path: w scales for the lower
            # classes, first xT piece, then upper classes, then the rest.
            # sxe batches are placed in the ACT queue right before the
            # first exp that needs them.
            for g in range(2):
                emit_wsum(g)
            emit_xbig(0)
            emit_xbig(1)
            emit_sxe_batch(0)
            for dc in range(4):
                emit_wT_piece(dc, 0)
            for dc in range(4):
                emit_xT_piece(dc, 0)
            for g in range(2, 4):
                emit_wsum(g)
            emit_xbig(2)
            emit_xbig(3)
            emit_sxe_batch(1)
            for dc in range(4):
                emit_wT_piece(dc, 1)
            for g in range(4, 8):
                emit_wsum(g)
            for dc in range(4):
                emit_wT_piece(dc, 2)
            for dc in range(4):
                emit_wT_piece(dc, 3)
            for dc in range(4):
                emit_xT_piece(dc, 1)
            emit_xbig(4)
            emit_xbig(5)
            emit_xbig(6)
            emit_xbig(7)
            for dc in range(4):
                emit_xT_piece(dc, 2)
            for dc in range(4):
                emit_xT_piece(dc, 3)
            for i in range(8, 16):
                emit_xbig(i)
            # warm the PE on first-half slots
            for t in range(8):
                emit_slot(t, 0)
            # the whole gather/dot stream is independent of the matmul/exp
            # pipeline; emitting it here keeps the pool queue free of
            # waiting collectives (which would block the gathers behind
            # them) and lets the dots finish long before the corrections
            for t in range(RT):
                emit_gather(t)

            for t in range(RT):
                if t in (4, 8, 12, 16, 20, 24):
                    emit_sxe_batch(t // 4 + 1)
                emit_slot(t, 1)
                if t + 8 < RT:
                    emit_slot(t + 8, 0)
                if t in (20, 36, 52):
                    emit_quarter((t - 20) // 16)
            emit_quarter(3)

            # ---- combine halves and reduce to the scalar mean ----
            gg = T([P, 2 * RT], "gg")
            for q in range(4):
                nc.gpsimd.dma_start(out=gg[:, q * QB:(q + 1) * QB],
                                    in_=ar_outs[q][:, 0:QB])
                nc.gpsimd.dma_start(
                    out=gg[:, RT + q * QB:RT + (q + 1) * QB],
                    in_=ar_outs[q][:, QB:2 * QB])

            at = T([P, RT], "at")
            nc.vector.tensor_scalar_add(out=at[:], in0=gg[:, 0:RT],
                                        scalar1=-NPAD)
            lna = T([P, RT], "lna")
            nc.scalar.activation(out=lna[:], in_=at[:], func=Ln)
            nll = T([P, RT], "nll")
            nc.vector.scalar_tensor_tensor(out=nll[:], in0=lna[:], scalar=1.0,
                                           in1=gg[:, RT:2 * RT],
                                           op0=mult, op1=sub)
            rsum = T([P, 1], "rsum")
            nc.vector.reduce_sum(out=rsum[:], in_=nll[:],
                                 axis=mybir.AxisListType.X)
            pf = pmm.tile([P, 2048], f32, name="ps")
            nc.tensor.matmul(out=pf[:1, :1], lhsT=rsum[:, :1],
                             rhs=ones[:, :1], start=True, stop=True)
            res = T([1, 1], "res")
            nc.vector.tensor_scalar_mul(out=res[:], in0=pf[:1, :1],
                                        scalar1=1.0 / float(N))
            nc.gpsimd.dma_start(out=out_d[:, :], in_=res[:])

    nc.compile()
    return nc


def _get_nc():
    if "nc" not in _CACHE:
        _CACHE["nc"] = _build()
    return _CACHE["nc"]


def kernel(prev_output, weight, labels, **trace_kwargs):
    x = np.ascontiguousarray(prev_output, dtype=np.float32)
    xT = np.ascontiguousarray(x.T)
    w = np.ascontiguousarray(weight, dtype=np.float32)
    lab = np.asarray(labels).astype(np.int64)

    wpad = np.zeros((CPAD, D), dtype=np.float32)
    wpad[:C] = w

    in_maps = []
    for k in range(NCORES):
        lo = k * CS
        wshard = np.ascontiguousarray(wpad[lo:lo + CS])
        loc = (lab - lo).astype(np.int64)
        own = (loc >= 0) & (loc < CS)
        locc = np.clip(loc, 0, CS - 1).astype(np.int32)
        # row r = t*128 + p maps to [p, t]
        lab2 = np.ascontiguousarray(locc.reshape(RT, P).T)
        msk2 = np.ascontiguousarray(own.astype(np.float32).reshape(RT, P).T)
        in_maps.append({
            "x": x,
            "xT": xT,
            "w": wshard,
            "wT": np.ascontiguousarray(wshard.T),
            "lab": lab2,
            "msk": msk2,
        })

    nc = _get_nc()
    res = run_bass_kernel_spmd(nc, in_maps, core_ids=list(range(NCORES)),
                               **trace_kwargs)
    if trace_kwargs:
        _CACHE["last_results"] = res
    return np.float32(res.results[0]["out"].reshape(())[()])


if __name__ == "__main__":
    rng = np.random.default_rng(0)
    x = rng.standard_normal((N, D), dtype=np.float32)
    w = rng.standard_normal((C, D), dtype=np.float32) * 0.01
    lab = rng.integers(0, C, N)
    got = kernel(x, w, lab)
    xh = x / np.maximum(np.linalg.norm(x, axis=1, keepdims=True), 1e-12)
    wh = w / np.maximum(np.linalg.norm(w, axis=1, keepdims=True), 1e-12)
    cos = (xh @ wh.T).astype(np.float64)
    th = np.arccos(np.clip(cos[np.arange(N), lab], -1 + EPS, 1 - EPS))
    ml = np.cos(th + M_MARGIN)
    logits = cos * S_SCALE
    tgt = ml * S_SCALE
    lse = np.log(np.exp(logits).sum(1) - np.exp(logits[np.arange(N), lab])
                 + np.exp(tgt))
    want = (lse - tgt).mean()
    print("got", got, "want", want, "relerr", abs(got - want) / abs(want))


# revision 26
# speedup vs baseline: 1.0924x; 1.0924x over previous
"""ArcMarginLoss distributed Trainium2 kernel (8 NeuronCores, class-sharded).

Math (equivalent to the reference, no arccos needed):
  x_hat = x / max(||x||, eps);  w_hat = w / max(||w||, eps)
  cos[i,c] = x_hat[i] . w_hat[c]
  For the label class only: m_i = cos(arccos(clip(c_i)) + M)
                                = clip(c_i)*cos(M) - sin(M)*sqrt(1-clip(c_i)^2)
  logits = S*cos except S*m_i at the label
  nll_i = logsumexp_c(logits[i]) - S*m_i
        = ln( sum_c exp(S*cos[i,c]) - exp(S*c_i) + exp(S*m_i) ) - S*m_i
  out = mean_i nll_i
S*cos is in [-16, 16] so no max-subtraction is needed for a stable sum-exp.

Distribution: classes padded 32000 -> 32768, 4096 per core.  Each core
computes its local sum-exp plus its owned rows' correction terms; two
[128,64] f32 AllReduces combine A[i] (sum-exp) and B[i] = S*m_i, then every
core computes mean(ln(A - 768) - B).

Implementation notes:
- matmul runs in fp8e4 with perf_mode=DoubleRow (2 fp8 weights per PE cell,
  ~2x bf16 throughput at free-dim 512): x is cast raw f32->fp8 (values
  ~N(0,1)); w is normalized and scaled by 32 before the fp8 cast so the
  values sit in fp8's normal range.  The psum accumulates fp32 and exp
  folds S/(32*||x||) in as a per-row activation scale, so overall rel-err
  stays ~2e-5.
- both x and w arrive ALSO transposed from the host ([D, N] / [D, CS]
  layout views) so the kernel needs no on-chip transposes at all.  The
  per-class norm scale is computed from the row-major w, bounced through
  DRAM, and broadcast-read back to all 128 partitions ([P, CS] tile), so
  the scale-and-cast of transposed w is a plain DVE multiply.
- the label-cosine path gathers raw w rows (indirect DMA) + the per-class
  scale from the DRAM bounce buffer, and dots against the row-major f32 x
  tiles on the DVE.
- sum-exp per 128-row tile: two [128, 2048] psum slots, ACT exp with
  accum_out; on every 4th row-tile the first slot is instead computed on
  the DVE with a Schraudolph fast-exp (scale+bias -> int32 -> bitcast f32
  -> accum), balancing the ACT and DVE engine loads.
- the activation-table cache is pre-seeded so Ln and Exp both resolve to
  the combined natural_log_exp set (one table load instead of ~28).
"""

import math
import sys

sys.path.insert(0, "/opt/trn_rl_repo")

import numpy as np

from concourse import bacc, bass, mybir, tile
from concourse.bass_utils import run_bass_kernel_spmd

f32 = mybir.dt.float32
bf16 = mybir.dt.bfloat16
fp8 = mybir.dt.float8e4
i32 = mybir.dt.int32

N, D, C = 8192, 512, 32000
NCORES = 8
CPAD = 32768            # padded class count (8 * 4096)
CS = CPAD // NCORES     # classes per core
P = 128                 # partitions
RT = N // P             # row tiles (64)
WGS = 8                 # w sumsq groups (512 classes each)
HB = RT // 2            # half of the row-tile columns
NPAD = float(CPAD - C)  # zero-pad classes, each contributes exp(0)=1

S_SCALE = 16.0
M_MARGIN = 0.2
EPS = 1e-7
COS_M = math.cos(M_MARGIN)
LN_SIN_M = math.log(math.sin(M_MARGIN))
SS_FLOOR = 1e-24        # max(ss, floor) emulates torch F.normalize eps=1e-12
FP8S = 32.0             # scale factor on normalized w before the fp8 cast
LN_S_FP8S = math.log(S_SCALE / FP8S)
LN_FP8S = math.log(FP8S)
FEXP_A = 2.0 ** 23 / math.log(2.0)       # fast-exp multiplier
FEXP_B = (127.0 - 0.0430) * 2.0 ** 23    # fast-exp bias (Schraudolph)
FE_EVERY = 4            # every FE_EVERY-th row-tile's first slot on the DVE

_CACHE = {}


def _patch_act_tables():
    """Make Ln and Exp resolve to the combined natural_log_exp set so the
    table-load pass emits one load instead of thrashing between sets."""
    from concourse.hw_specs import get_activation_tables

    Exp = mybir.ActivationFunctionType.Exp
    Ln = mybir.ActivationFunctionType.Ln
    tabs = get_activation_tables("gen3")   # cached dict, mutate in place
    combined = [n for n, fns in tabs.items() if Exp in fns and Ln in fns]
    if not combined:
        return
    keep = combined[0]
    for name, fns in tabs.items():
        if name != keep:
            fns.discard(Exp)
            fns.discard(Ln)


def _build(ncores=NCORES):
    _patch_act_tables()
    nc = bacc.Bacc("TRN2", target_bir_lowering=False, debug=False,
                   num_devices=ncores)
    x_d = nc.dram_tensor("x", [N, D], f32, kind="ExternalInput")
    xT_d = nc.dram_tensor("xT", [D, N], f32, kind="ExternalInput")
    w_d = nc.dram_tensor("w", [CS, D], f32, kind="ExternalInput")
    wT_d = nc.dram_tensor("wT", [D, CS], f32, kind="ExternalInput")
    lab_d = nc.dram_tensor("lab", [P, RT], i32, kind="ExternalInput")
    msk_d = nc.dram_tensor("msk", [P, RT], f32, kind="ExternalInput")
    out_d = nc.dram_tensor("out", [1, 1], f32, kind="ExternalOutput")

    mult = mybir.AluOpType.mult
    add = mybir.AluOpType.add
    sub = mybir.AluOpType.subtract
    amax = mybir.AluOpType.max
    amin = mybir.AluOpType.min
    Exp = mybir.ActivationFunctionType.Exp
    Ln = mybir.ActivationFunctionType.Ln
    DR = mybir.MatmulPerfMode.DoubleRow

    with tile.TileContext(nc) as tc:
        with tc.tile_pool(name="persist", bufs=1) as persist, \
             tc.tile_pool(name="dram", bufs=1, space="DRAM") as dram, \
             tc.tile_pool(name="xts", bufs=3) as xts, \
             tc.tile_pool(name="wts", bufs=4) as wts, \
             tc.tile_pool(name="xrow", bufs=4) as xrow, \
             tc.tile_pool(name="xbig", bufs=2) as xbig, \
             tc.tile_pool(name="wsum", bufs=3) as wsum, \
             tc.tile_pool(name="ej", bufs=2) as ejp, \
             tc.tile_pool(name="junk", bufs=2) as junkp, \
             tc.tile_pool(name="small", bufs=6) as small, \
             tc.tile_pool(name="pmm", bufs=2, space="PSUM") as pmm:

            # --- persistent tiles ---
            def T(shape, name, dtype=f32):
                return persist.tile(shape, dtype, name=name)

            ones = T([P, 1], "ones")
            nc.vector.memset(ones[:], 1.0)
            ln_fp8s_c = T([P, 1], "ln_fp8s_c")
            nc.vector.memset(ln_fp8s_c[:], LN_FP8S)
            ln_s_fp8s_c = T([P, 1], "ln_s_fp8s_c")
            nc.vector.memset(ln_s_fp8s_c[:], LN_S_FP8S)
            ln_sin_m_c = T([P, 1], "ln_sin_m_c")
            nc.vector.memset(ln_sin_m_c[:], LN_SIN_M)

            labs = T([P, RT], "labs", dtype=i32)
            nc.gpsimd.dma_start(out=labs[:], in_=lab_d[:, :])
            msks = T([P, RT], "msks")
            nc.gpsimd.dma_start(out=msks[:], in_=msk_d[:, :])

            xT8 = T([P, 4, N], "xT8", dtype=fp8)       # 32KB/part
            whT8 = T([P, 4, CS], "whT8", dtype=fp8)    # 16KB/part
            wscb = T([P, CS], "wscb")                  # 16KB/part

            ssx = T([P, RT], "ssx")
            sxe = T([P, RT], "sxe")      # S / (32*||x_r||), exp scale
            sxf = T([P, RT], "sxf")      # sxe * 2^23/ln2, fast-exp scale
            ssw = T([P, WGS * 4], "ssw")
            wsc = T([P, WGS * 4], "wsc")
            dotg = T([P, RT], "dotg")
            ssg = T([P, RT], "ssg")      # ||w_label||^2 from gathered rows
            accAB = T([P, 2 * RT], "accAB")
            sumexp = T([P, RT], "sumexp")
            cdot = T([P, RT], "cdot")
            ctl = T([P, RT], "ctl")
            marg = T([P, RT], "marg")
            aloc = T([P, RT], "aloc")
            bloc = T([P, RT], "bloc")

            wsc_dram = dram.tile([CS, 1], f32)
            QB = RT // 4
            ar_ins = [dram.tile([P, 2 * QB], f32, name=f"ar_in{q}")
                      for q in range(4)]
            ar_outs = [dram.tile([P, 2 * QB], f32, name=f"ar_out{q}")
                       for q in range(4)]

            # ---- x transposed pieces: [D, N] f32 -> fp8, no transpose ----
            def emit_xT_piece(dc, cc):
                xtp = xts.tile([P, 2048], f32, name="xtp")
                nc.sync.dma_start(
                    out=xtp[:],
                    in_=xT_d[dc * P:(dc + 1) * P, cc * 2048:(cc + 1) * 2048])
                with tc.high_priority():
                    nc.vector.tensor_copy(
                        out=xT8[:, dc, cc * 2048:(cc + 1) * 2048],
                        in_=xtp[:])

            # ---- w norm scales: sumsq from row-major w, bounce through
            #      DRAM, broadcast back to all partitions ----
            def emit_wsum(g):
                wt = wsum.tile([P, 4, D], f32, name="wt")
                # class c = g*512 + p*4 + a  -> 8KB contiguous/partition
                nc.sync.dma_start(
                    out=wt[:],
                    in_=w_d[g * 4 * P:(g + 1) * 4 * P, :].rearrange(
                        "(p a) d -> p a d", a=4))
                with tc.high_priority():
                    for a in range(4):
                        j = g * 4 + a
                        wjk = junkp.tile([P, D], f32, name="junk")
                        nc.vector.scalar_tensor_tensor(
                            out=wjk[:], in0=wt[:, a], scalar=1.0,
                            in1=wt[:, a],
                            op0=mult, op1=mult, accum_out=ssw[:, j:j + 1])
                    gsl = slice(g * 4, (g + 1) * 4)
                    wssc = small.tile([P, 4], f32, name="wssc")
                    nc.vector.tensor_scalar_max(out=wssc[:], in0=ssw[:, gsl],
                                                scalar1=SS_FLOOR)
                    wlns = small.tile([P, 4], f32, name="wlns")
                    nc.scalar.activation(out=wlns[:], in_=wssc[:], func=Ln)
                    # wsc = 32 * ss^-0.5
                    nc.scalar.activation(out=wsc[:, gsl], in_=wlns[:],
                                         func=Exp, scale=-0.5,
                                         bias=ln_fp8s_c[:, :1])
                nc.gpsimd.dma_start(
                    out=wsc_dram[g * 4 * P:(g + 1) * 4 * P, 0].rearrange(
                        "(p a) -> p a", a=4),
                    in_=wsc[:, gsl])
                nc.gpsimd.dma_start(
                    out=wscb[:, g * 512:(g + 1) * 512],
                    in_=wsc_dram[g * 512:(g + 1) * 512, 0].unsqueeze(
                        0).to_broadcast((P, 512)))

            # ---- transposed w: scale (free-dim bcast tile) + fp8 cast ----
            def emit_wT_piece(dc, h):
                wtp = wts.tile([P, 1024], f32, name="wtp")
                nc.sync.dma_start(
                    out=wtp[:],
                    in_=wT_d[dc * P:(dc + 1) * P, h * 1024:(h + 1) * 1024])
                with tc.high_priority():
                    nc.vector.tensor_tensor(
                        out=whT8[:, dc, h * 1024:(h + 1) * 1024],
                        in0=wtp[:], in1=wscb[:, h * 1024:(h + 1) * 1024],
                        op=mult)

            # ---- x row sumsq for ||x||, 4 row-tiles per DMA ----
            def emit_xbig(i):
                xb = xbig.tile([P, 4, D], f32, name="xb")
                # row r = i*512 + a*128 + p: sub-tile a = row-tile 4i+a
                nc.sync.dma_start(
                    out=xb[:],
                    in_=x_d[i * 4 * P:(i + 1) * 4 * P, :].rearrange(
                        "(a p) d -> p a d", a=4))
                with tc.high_priority():
                    for a in range(4):
                        xjk = junkp.tile([P, D], f32, name="junk")
                        nc.vector.scalar_tensor_tensor(
                            out=xjk[:], in0=xb[:, a], scalar=1.0,
                            in1=xb[:, a],
                            op0=mult, op1=mult,
                            accum_out=ssx[:, 4 * i + a:4 * i + a + 1])

            def emit_sxe_batch(b):
                with tc.high_priority():
                    bs = slice(b * 8, (b + 1) * 8)
                    xln = small.tile([P, 8], f32, name="xln")
                    nc.scalar.activation(out=xln[:], in_=ssx[:, bs], func=Ln)
                    nc.scalar.activation(out=sxe[:, bs], in_=xln[:],
                                         func=Exp, scale=-0.5,
                                         bias=ln_s_fp8s_c[:, :1])
                    nc.vector.tensor_scalar_mul(out=sxf[:, bs],
                                                in0=sxe[:, bs],
                                                scalar1=FEXP_A)

            # ---- label gather (raw w rows) + dot + row sumsq ----
            def emit_gather(t):
                wg_t = small.tile([P, D], f32, name="wg")
                nc.gpsimd.indirect_dma_start(
                    out=wg_t[:], out_offset=None, in_=w_d[:, :],
                    in_offset=bass.IndirectOffsetOnAxis(
                        ap=labs[:, t:t + 1], axis=0))
                xt2 = xrow.tile([P, D], f32, name="xt2")
                nc.sync.dma_start(out=xt2[:], in_=x_d[t * P:(t + 1) * P, :])
                gjk = junkp.tile([P, D], f32, name="gjunk")
                nc.vector.scalar_tensor_tensor(
                    out=gjk[:], in0=wg_t[:], scalar=1.0, in1=xt2[:],
                    op0=mult, op1=mult, accum_out=dotg[:, t:t + 1])
                gj2 = junkp.tile([P, D], f32, name="gjunk")
                nc.vector.scalar_tensor_tensor(
                    out=gj2[:], in0=wg_t[:], scalar=1.0, in1=wg_t[:],
                    op0=mult, op1=mult, accum_out=ssg[:, t:t + 1])

            # ---- per-row-tile slots: 8 DR matmuls + exp-accumulate ----
            def emit_slot(t, half):
                ps = pmm.tile([P, 2048], f32, name="ps")
                rs = slice(t * P, (t + 1) * P)
                for c in range(4):
                    cg = half * 2048 + c * 512
                    for k in range(2):
                        nc.tensor.matmul(
                            out=ps[:, c * 512:(c + 1) * 512],
                            lhsT=xT8[:, 2 * k:2 * k + 2, rs],
                            rhs=whT8[:, 2 * k:2 * k + 2, cg:cg + 512],
                            start=(k == 0), stop=(k == 1),
                            perf_mode=DR)
                acol = accAB[:, 2 * t + half:2 * t + half + 1]
                if False and half == 0 and t % FE_EVERY == 2:
                    # DVE fast-exp for this slot (engine balancing); high
                    # priority so it never queues behind the dot stream
                    with tc.high_priority(offset=400):
                        bt = bitsp.tile([P, 2048], i32, name="bt")
                        nc.vector.tensor_scalar(
                            out=bt[:], in0=ps[:],
                            scalar1=sxf[:, t:t + 1], scalar2=FEXP_B,
                            op0=mult, op1=add)
                        jb = bitsp.tile([P, 2048], bf16, name="jb")
                        nc.vector.tensor_scalar(
                            out=jb[:], in0=bt[:].bitcast(f32),
                            scalar1=1.0, scalar2=0.0, op0=mult, op1=add,
                            accum_out=acol)
                else:
                    ej = ejp.tile([P, 2048], bf16, name="ej")
                    nc.scalar.activation(
                        out=ej[:], in_=ps[:], func=Exp,
                        scale=sxe[:, t:t + 1], accum_out=acol)

            # ---- corrections + allreduce for one quarter of the rows ----
            def emit_quarter(q):
                cl = slice(q * QB, (q + 1) * QB)
                a2 = accAB[:, 2 * q * QB:2 * (q + 1) * QB].rearrange(
                    "p (t two) -> p two t", two=2)
                nc.vector.tensor_tensor(out=sumexp[:, cl], in0=a2[:, 0],
                                        in1=a2[:, 1], op=add)
                # gsc = ||w_lab||^-1;  cos_i = dotg * gsc * sxe * 32/S
                gssc = small.tile([P, QB], f32, name="gssc")
                nc.vector.tensor_scalar_max(out=gssc[:], in0=ssg[:, cl],
                                            scalar1=SS_FLOOR)
                glns = small.tile([P, QB], f32, name="glns")
                nc.scalar.activation(out=glns[:], in_=gssc[:], func=Ln)
                gsc = small.tile([P, QB], f32, name="gsc")
                nc.scalar.activation(out=gsc[:], in_=glns[:], func=Exp,
                                     scale=-0.5)
                cd = cdot[:, cl]
                nc.vector.tensor_tensor(out=cd, in0=dotg[:, cl],
                                        in1=gsc[:], op=mult)
                nc.vector.scalar_tensor_tensor(
                    out=cd, in0=cd, scalar=FP8S / S_SCALE,
                    in1=sxe[:, cl], op0=mult, op1=mult)
                nc.vector.tensor_scalar(out=ctl[:, cl], in0=cd,
                                        scalar1=(-1.0 + EPS),
                                        scalar2=(1.0 - EPS),
                                        op0=amax, op1=amin)
                negc2 = small.tile([P, QB], f32, name="negc2")
                nc.vector.scalar_tensor_tensor(out=negc2[:], in0=ctl[:, cl],
                                               scalar=-1.0, in1=ctl[:, cl],
                                               op0=mult, op1=mult)
                uu = small.tile([P, QB], f32, name="uu")
                nc.vector.tensor_scalar_add(out=uu[:], in0=negc2[:],
                                            scalar1=1.0)
                lnu = small.tile([P, QB], f32, name="lnu")
                nc.scalar.activation(out=lnu[:], in_=uu[:], func=Ln)
                sinsq = small.tile([P, QB], f32, name="sinsq")
                nc.scalar.activation(out=sinsq[:], in_=lnu[:], func=Exp,
                                     scale=0.5, bias=ln_sin_m_c[:, :1])
                nc.vector.scalar_tensor_tensor(out=marg[:, cl],
                                               in0=ctl[:, cl],
                                               scalar=COS_M, in1=sinsq[:],
                                               op0=mult, op1=sub)
                e1 = small.tile([P, QB], f32, name="e1")
                nc.scalar.activation(out=e1[:], in_=marg[:, cl], func=Exp,
                                     scale=S_SCALE)
                e2 = small.tile([P, QB], f32, name="e2")
                nc.scalar.activation(out=e2[:], in_=ctl[:, cl], func=Exp,
                                     scale=S_SCALE)
                d12 = small.tile([P, QB], f32, name="d12")
                nc.vector.scalar_tensor_tensor(out=d12[:], in0=e1[:],
                                               scalar=1.0, in1=e2[:],
                                               op0=mult, op1=sub)
                corr = small.tile([P, QB], f32, name="corr")
                nc.vector.tensor_tensor(out=corr[:], in0=d12[:],
                                        in1=msks[:, cl], op=mult)
                nc.vector.tensor_tensor(out=aloc[:, cl], in0=sumexp[:, cl],
                                        in1=corr[:], op=add)
                nc.vector.scalar_tensor_tensor(out=bloc[:, cl],
                                               in0=marg[:, cl],
                                               scalar=S_SCALE,
                                               in1=msks[:, cl],
                                               op0=mult, op1=mult)
                ar_i, ar_o = ar_ins[q], ar_outs[q]
                nc.gpsimd.dma_start(out=ar_i[:, 0:QB], in_=aloc[:, cl])
                nc.gpsimd.dma_start(out=ar_i[:, QB:2 * QB], in_=bloc[:, cl])
                nc.gpsimd.collective_compute(
                    "AllReduce", add,
                    replica_groups=[list(range(ncores))],
                    ins=[ar_i[:].opt()], outs=[ar_o[:].opt()])

            # ================= emission schedule =================
            # DVE order tracks the critical path: w scales for the lower
            # classes, first xT piece, then upper classes, then the rest.
            # sxe batches are placed in the ACT queue right before the
            # first exp that needs them.
            for g in range(2):
                emit_wsum(g)
            emit_xbig(0)
            emit_xbig(1)
            emit_sxe_batch(0)
            for dc in range(4):
                emit_wT_piece(dc, 0)
            for dc in range(4):
                emit_xT_piece(dc, 0)
            for g in range(2, 4):
                emit_wsum(g)
            emit_xbig(2)
            emit_xbig(3)
            emit_sxe_batch(1)
            for dc in range(4):
                emit_wT_piece(dc, 1)
            for g in range(4, 8):
                emit_wsum(g)
            for dc in range(4):
                emit_wT_piece(dc, 2)
            for dc in range(4):
                emit_wT_piece(dc, 3)
            for dc in range(4):
                emit_xT_piece(dc, 1)
            emit_xbig(4)
            emit_xbig(5)
            emit_xbig(6)
            emit_xbig(7)
            for dc in range(4):
                emit_xT_piece(dc, 2)
            for dc in range(4):
                emit_xT_piece(dc, 3)
            for i in range(8, 16):
                emit_xbig(i)
            # warm the PE on first-half slots
            for t in range(8):
                emit_slot(t, 0)
            # the whole gather/dot stream is independent of the matmul/exp
            # pipeline; emitting it here keeps the pool queue free of
            # waiting collectives (which would block the gathers behind
            # them) and lets the dots finish long before the corrections
            for t in range(RT):
                emit_gather(t)

            for t in range(RT):
                if t in (4, 8, 12, 16, 20, 24):
                    emit_sxe_batch(t // 4 + 1)
                emit_slot(t, 1)
                if t + 8 < RT:
                    emit_slot(t + 8, 0)
                if t in (20, 36, 52):
                    emit_quarter((t - 20) // 16)
            emit_quarter(3)

            # ---- combine halves and reduce to the scalar mean ----
            gg = T([P, 2 * RT], "gg")
            for q in range(4):
                nc.gpsimd.dma_start(out=gg[:, q * QB:(q + 1) * QB],
                                    in_=ar_outs[q][:, 0:QB])
                nc.gpsimd.dma_start(
                    out=gg[:, RT + q * QB:RT + (q + 1) * QB],
                    in_=ar_outs[q][:, QB:2 * QB])

            at = T([P, RT], "at")
            nc.vector.tensor_scalar_add(out=at[:], in0=gg[:, 0:RT],
                                        scalar1=-NPAD)
            lna = T([P, RT], "lna")
            nc.scalar.activation(out=lna[:], in_=at[:], func=Ln)
            nll = T([P, RT], "nll")
            nc.vector.scalar_tensor_tensor(out=nll[:], in0=lna[:], scalar=1.0,
                                           in1=gg[:, RT:2 * RT],
                                           op0=mult, op1=sub)
            rsum = T([P, 1], "rsum")
            nc.vector.reduce_sum(out=rsum[:], in_=nll[:],
                                 axis=mybir.AxisListType.X)
            pf = pmm.tile([P, 2048], f32, name="ps")
            nc.tensor.matmul(out=pf[:1, :1], lhsT=rsum[:, :1],
                             rhs=ones[:, :1], start=True, stop=True)
            res = T([1, 1], "res")
            nc.vector.tensor_scalar_mul(out=res[:], in0=pf[:1, :1],
                                        scalar1=1.0 / float(N))
            nc.gpsimd.dma_start(out=out_d[:, :], in_=res[:])

    nc.compile()
    return nc


def _get_nc():
    if "nc" not in _CACHE:
        _CACHE["nc"] = _build()
    return _CACHE["nc"]


def kernel(prev_output, weight, labels, **trace_kwargs):
    x = np.ascontiguousarray(prev_output, dtype=np.float32)
    xT = np.ascontiguousarray(x.T)
    w = np.ascontiguousarray(weight, dtype=np.float32)
    lab = np.asarray(labels).astype(np.int64)

    wpad = np.zeros((CPAD, D), dtype=np.float32)
    wpad[:C] = w

    in_maps = []
    for k in range(NCORES):
        lo = k * CS
        wshard = np.ascontiguousarray(wpad[lo:lo + CS])
        loc = (lab - lo).astype(np.int64)
        own = (loc >= 0) & (loc < CS)
        locc = np.clip(loc, 0, CS - 1).astype(np.int32)
        # row r = t*128 + p maps to [p, t]
        lab2 = np.ascontiguousarray(locc.reshape(RT, P).T)
        msk2 = np.ascontiguousarray(own.astype(np.float32).reshape(RT, P).T)
        in_maps.append({
            "x": x,
            "xT": xT,
            "w": wshard,
            "wT": np.ascontiguousarray(wshard.T),
            "lab": lab2,
            "msk": msk2,
        })

    nc = _get_nc()
    res = run_bass_kernel_spmd(nc, in_maps, core_ids=list(range(NCORES)),
                               **trace_kwargs)
    if trace_kwargs:
        _CACHE["last_results"] = res
    return np.float32(res.results[0]["out"].reshape(())[()])


if __name__ == "__main__":
    rng = np.random.default_rng(0)
    x = rng.standard_normal((N, D), dtype=np.float32)
    w = rng.standard_normal((C, D), dtype=np.float32) * 0.01
    lab = rng.integers(0, C, N)
    got = kernel(x, w, lab)
    xh = x / np.maximum(np.linalg.norm(x, axis=1, keepdims=True), 1e-12)
    wh = w / np.maximum(np.linalg.norm(w, axis=1, keepdims=True), 1e-12)
    cos = (xh @ wh.T).astype(np.float64)
    th = np.arccos(np.clip(cos[np.arange(N), lab], -1 + EPS, 1 - EPS))
    ml = np.cos(th + M_MARGIN)
    logits = cos * S_SCALE
    tgt = ml * S_SCALE
    lse = np.log(np.exp(logits).sum(1) - np.exp(logits[np.arange(N), lab])
                 + np.exp(tgt))
    want = (lse - tgt).mean()
    print("got", got, "want", want, "relerr", abs(got - want) / abs(want))
